# revision 1
# baseline (speedup 1.0000x reference)
"""Causal multi-head attention (B=32,T=512,C=1024,H=16,D=64) on 8 TRN2 cores.

Strategy: pure data-parallel over the batch axis (4 batches per core, no
collectives). Per core, per batch:
  - x^T [C,T] arrives pre-transposed from the host (layout prep only).
  - Q^T [HD,T] and V [T,HD] computed with bf16 matmuls (fp32 PSUM); K^T is
    written into per-head zero-padded [128,T] tiles so every PE matmul runs
    in the full 128x128 array mode (no tiling-mode switches/drains).
  - scores^T [s,t] blocks computed directly on PE (only the causal lower
    triangle of [T,T], packed into a [128,1280] PSUM tile per head).
  - softmax without max-subtraction: scores here are bounded (|s|<~3) so
    exp is safe in fp32; masked entries are zeroed by multiplying the
    exp'd diagonal blocks with a 0/1 triangular mask.
  - attn@V with a ones-augmented V column producing the softmax row-sums
    in the same matmul; normalize via DVE reciprocal + per-partition scale.
  - head-concat transpose via one batched DMA-transpose per t-chunk;
    final projection with bias folded into a K=128 matmul; fp32 output.
"""

import sys

if "/opt/trn_rl_repo" not in sys.path:
    sys.path.insert(0, "/opt/trn_rl_repo")

import numpy as np
import ml_dtypes

B, T, C = 32, 512, 1024
H, D = 16, 64
HD = H * D
NCORES = 8
B_LOC = B // NCORES

_CACHE = {}


def build_nc(b_loc=B_LOC):
    import concourse.mybir as mybir
    from concourse import bacc
    from concourse.bass import ds, ts
    from concourse.tile import TileContext

    f32 = mybir.dt.float32
    bf16 = mybir.dt.bfloat16
    AF = mybir.ActivationFunctionType

    KO = C // 128  # 8 contraction chunks
    MO = HD // 128  # 8 output-row chunks
    TCH = T // 128  # 4 t-chunks
    SCALE = 1.0 / float(np.sqrt(C))

    # scores^T causal packing: s-chunk j covers t in [128j, T), width T-128j.
    # Packed into one PSUM tile [128, 1280] so no matmul output crosses a
    # 2KB bank boundary: j0@[0,512) bank0, j1@[512,896) bank1,
    # j3@[896,1024) bank1, j2@[1024,1280) bank2.
    widths = [T - 128 * j for j in range(TCH)]
    off = [0, 512, 1024, 896]
    PACK = 1280

    nc = bacc.Bacc("TRN2", target_bir_lowering=False)
    xT = nc.dram_tensor("xT", [b_loc, C, T], bf16, kind="ExternalInput")
    wq = nc.dram_tensor("wq", [C, HD], bf16, kind="ExternalInput")
    wk = nc.dram_tensor("wk", [C, HD], bf16, kind="ExternalInput")
    wv = nc.dram_tensor("wv", [C, HD], bf16, kind="ExternalInput")
    wp = nc.dram_tensor("wp", [C, C], bf16, kind="ExternalInput")
    bp = nc.dram_tensor("bp", [1, C], bf16, kind="ExternalInput")
    mask = nc.dram_tensor("mask", [128, 128], bf16, kind="ExternalInput")
    out = nc.dram_tensor("out", [b_loc, T, C], f32, kind="ExternalOutput")

    with TileContext(nc) as tc:
        with (
            tc.tile_pool(name="weights", bufs=1) as wpool,
            tc.tile_pool(name="acts", bufs=2) as xpool,
            tc.tile_pool(name="attn", bufs=4) as apool,
            tc.tile_pool(name="small", bufs=8) as spool,
            tc.tile_pool(name="ons", bufs=3) as onpool,
            tc.tile_pool(name="outs", bufs=2) as opool,
            tc.tile_pool(name="psS", bufs=2, space="PSUM") as psA,
            tc.tile_pool(name="ps1", bufs=2, space="PSUM") as psB,
        ):
            # ---- persistent weights ----
            # DMA order matters for the pipeline head: first batch's x^T and
            # the early-needed weight chunks go first, chunk by chunk, so the
            # first QKV matmuls start as soon as their operands land.
            wq_sb = wpool.tile([128, KO, HD], bf16, name="wq_sb")
            wk_sb = wpool.tile([128, KO, HD], bf16, name="wk_sb")
            wv_sb = wpool.tile([128, KO, HD], bf16, name="wv_sb")
            wp_sb = wpool.tile([128, KO, C], bf16, name="wp_sb")
            # batch 0's x^T goes first so QKV can start immediately
            xT0_sb = xpool.tile([128, KO, T], bf16, name="xT0_sb", tag="xT")
            # tiny inputs first: the bias-broadcast matmuls sit early in the
            # in-order PE queue, so bp must not land behind 13MB of weights
            bp1_sb = wpool.tile([1, C], bf16, name="bp1_sb")
            nc.sync.dma_start(out=bp1_sb, in_=bp[:])
            mask_sb = wpool.tile([128, 128], bf16, name="mask_sb")
            nc.sync.dma_start(out=mask_sb, in_=mask[:])
            # interleave so the first Q-projection chain can start ASAP
            for k in range(KO):
                nc.sync.dma_start(out=xT0_sb[:, k, :], in_=xT[0, ds(128 * k, 128), :])
                nc.sync.dma_start(out=wq_sb[:, k, :], in_=wq[ds(128 * k, 128), :])
            for w_sb, w_dram in ((wk_sb, wk), (wv_sb, wv)):
                for k in range(KO):
                    nc.sync.dma_start(
                        out=w_sb[:, k, :], in_=w_dram[ds(128 * k, 128), :]
                    )
            nc.sync.dma_start(
                out=wp_sb, in_=wp[:].rearrange("(ko p) n -> p ko n", p=128)
            )
            # bias broadcast to all 128 partitions, f32, built once
            ones1_sb = wpool.tile([1, 128], bf16, name="ones1_sb")
            nc.gpsimd.memset(ones1_sb, 1.0)
            bias_bc = wpool.tile([128, C], f32, name="bias_bc")
            for half in range(2):
                psb = psB.tile([128, 512], f32, name="psb", tag="ps1")
                nc.tensor.matmul(
                    psb, ones1_sb, bp1_sb[:, ts(half, 512)], start=True, stop=True
                )
                nc.vector.tensor_copy(out=bias_bc[:, ts(half, 512)], in_=psb)
            # K^T in per-head zero-padded layout; two persistent slots for
            # cross-batch overlap. Zero halves are written once, ever.
            kT2_tiles = []
            for slot in range(2):
                t_ = wpool.tile([128, H, T], bf16, name=f"kT2_{slot}")
                nc.gpsimd.memset(t_, 0.0)
                kT2_tiles.append(t_)

            for b in range(b_loc):
                kT2 = kT2_tiles[b % 2]
                # ---- load x^T for this batch ----
                if b == 0:
                    xT_sb = xT0_sb
                else:
                    xT_sb = xpool.tile([128, KO, T], bf16, name="xT_sb", tag="xT")
                    for k in range(KO):
                        nc.sync.dma_start(
                            out=xT_sb[:, k, :], in_=xT[b, ds(128 * k, 128), :]
                        )

                # ---- Q^T projection: [HD, T] ----
                qT_sb = xpool.tile([128, MO, T], bf16, name="qT_sb", tag="qT")
                for m in range(MO):
                    ps = psB.tile([128, T], f32, name="ps_q", tag="ps1")
                    for k in range(KO):
                        nc.tensor.matmul(
                            ps,
                            wq_sb[:, k, ts(m, 128)],
                            xT_sb[:, k, :],
                            start=(k == 0),
                            stop=(k == KO - 1),
                        )
                    nc.scalar.copy(out=qT_sb[:, m, :], in_=ps)

                # ---- K^T projection into zero-padded per-head tiles ----
                for m in range(MO):
                    ps = psB.tile([128, T], f32, name="ps_k", tag="ps1")
                    for k in range(KO):
                        nc.tensor.matmul(
                            ps,
                            wk_sb[:, k, ts(m, 128)],
                            xT_sb[:, k, :],
                            start=(k == 0),
                            stop=(k == KO - 1),
                        )
                    # head 2m -> partitions 0:64, head 2m+1 -> partitions 64:128
                    nc.vector.tensor_copy(out=kT2[0:64, 2 * m, :], in_=ps[0:64, :])
                    nc.vector.tensor_copy(
                        out=kT2[64:128, 2 * m + 1, :], in_=ps[64:128, :]
                    )

                # ---- V: [s, h, 65] with ones column at d=64 ----
                v_sb = xpool.tile([128, TCH, H, 65], bf16, name="v_sb", tag="v")
                nc.vector.memset(v_sb[:, :, :, 64:65], 1.0)
                for i in range(TCH):
                    for half in range(2):
                        ps = psB.tile([128, 512], f32, name="ps_v", tag="ps1")
                        for k in range(KO):
                            nc.tensor.matmul(
                                ps,
                                xT_sb[:, k, ts(i, 128)],
                                wv_sb[:, k, ts(half, 512)],
                                start=(k == 0),
                                stop=(k == KO - 1),
                            )
                        nc.scalar.copy(
                            out=v_sb[:, i, 8 * half : 8 * half + 8, 0:64],
                            in_=ps.rearrange("p (h d) -> p h d", d=64),
                        )

                # ---- attention, one head at a time ----
                outT_sb = opool.tile([128, MO, T], bf16, name="outT_sb", tag="outT")
                on_tiles = [
                    onpool.tile([128, HD], bf16, name=f"on{i}", tag=f"on{i % 2}")
                    for i in range(TCH)
                ]
                for h in range(H):
                    pair, pb = h // 2, 64 * (h % 2)
                    # scores^T blocks (only the causal triangle), K=128 padded
                    psS = psA.tile([128, PACK], f32, name="psS", tag="psS")
                    for j in range(TCH):
                        nc.tensor.matmul(
                            psS[:, ds(off[j], widths[j])],
                            kT2[:, h, ts(j, 128)],
                            qT_sb[:, pair, ds(128 * j, widths[j])],
                            start=True,
                            stop=True,
                        )
                    aT = apool.tile([128, PACK], bf16, name="aT", tag="aT")
                    nc.scalar.activation(aT, psS, AF.Exp, scale=SCALE)
                    # zero the masked (s>t) part of the diagonal blocks
                    for j in range(TCH):
                        nc.gpsimd.tensor_mul(
                            aT[:, ds(off[j], 128)], aT[:, ds(off[j], 128)], mask_sb
                        )
                    # attn @ [V | 1]: out [t, 65]; col 64 = softmax row-sum
                    for i in range(TCH):
                        psAV = psB.tile([128, 65], f32, name="psAV", tag="ps1")
                        for j in range(i + 1):
                            nc.tensor.matmul(
                                psAV,
                                aT[:, ds(off[j] + 128 * (i - j), 128)],
                                v_sb[:, j, h, :],
                                start=(j == 0),
                                stop=(j == i),
                            )
                        rr = spool.tile([128, 1], f32, name="rr", tag="rr")
                        nc.vector.reciprocal(rr, psAV[:, 64:65])
                        nc.vector.tensor_scalar_mul(
                            on_tiles[i][:, ds(128 * pair + pb, 64)],
                            psAV[:, 0:64],
                            rr,
                        )
                # batched head-concat transpose: [t, hd] -> [hd, t] per t-chunk
                for i in range(TCH):
                    nc.scalar.dma_start_transpose(
                        out=outT_sb[:, :, ts(i, 128)], in_=on_tiles[i]
                    )

                # ---- final projection; bias added during PSUM evacuation ----
                for i in range(TCH):
                    out_sb = opool.tile([128, C], f32, name="out_sb", tag="out_sb")
                    for half in range(2):
                        # use the big scores-pool slots: keeps the small pool
                        # free for AV/QKV chains during phase overlap
                        psF = psA.tile([128, 512], f32, name="psF", tag="psS")
                        for k in range(MO):
                            nc.tensor.matmul(
                                psF,
                                outT_sb[:, k, ts(i, 128)],
                                wp_sb[:, k, ts(half, 512)],
                                start=(k == 0),
                                stop=(k == MO - 1),
                            )
                        nc.vector.tensor_add(
                            out=out_sb[:, ts(half, 512)],
                            in0=psF,
                            in1=bias_bc[:, ts(half, 512)],
                        )
                    nc.sync.dma_start(out=out[b, ts(i, 128), :], in_=out_sb)

    nc.compile()
    return nc


def make_in_maps(x, wq, wk, wv, w_proj, b_proj, b_loc=B_LOC, ncores=NCORES):
    bf16 = ml_dtypes.bfloat16
    x = np.asarray(x, dtype=np.float32)
    # host-side layout prep (transpose / reshape / cast only)
    xT = np.ascontiguousarray(x.transpose(0, 2, 1)).astype(bf16)  # [B, C, T]
    wq2 = np.ascontiguousarray(
        np.asarray(wq, np.float32).transpose(1, 0, 2).reshape(C, HD)
    ).astype(bf16)
    wk2 = np.ascontiguousarray(
        np.asarray(wk, np.float32).transpose(1, 0, 2).reshape(C, HD)
    ).astype(bf16)
    wv2 = np.ascontiguousarray(
        np.asarray(wv, np.float32).transpose(1, 0, 2).reshape(C, HD)
    ).astype(bf16)
    wp2 = np.ascontiguousarray(np.asarray(w_proj, np.float32)).astype(bf16)
    bp2 = np.asarray(b_proj, np.float32).reshape(1, C).astype(bf16)
    # mask[p, f] = 1 where p <= f (valid: s_in <= t_in on diagonal blocks)
    m = np.triu(np.ones((128, 128), np.float32)).astype(bf16)
    in_maps = []
    for c in range(ncores):
        in_maps.append(
            {
                "xT": xT[c * b_loc : (c + 1) * b_loc],
                "wq": wq2,
                "wk": wk2,
                "wv": wv2,
                "wp": wp2,
                "bp": bp2,
                "mask": m,
            }
        )
    return in_maps


def kernel(x, wq, wk, wv, w_proj, b_proj, **run_kwargs):
    from concourse import bass_utils

    if "nc" not in _CACHE:
        _CACHE["nc"] = build_nc(B_LOC)
    nc = _CACHE["nc"]
    in_maps = make_in_maps(x, wq, wk, wv, w_proj, b_proj)
    res = bass_utils.run_bass_kernel_spmd(
        nc, in_maps, core_ids=list(range(NCORES)), **run_kwargs
    )
    outs = [r["out"] for r in res.results]
    full = np.concatenate(outs, axis=0).astype(np.float32)
    if run_kwargs:
        _CACHE["last_result"] = res
    return full



# revision 2
# speedup vs baseline: 1.1453x; 1.1453x over previous
"""Causal multi-head attention (B=32,T=512,C=1024,H=16,D=64) on 8 TRN2 cores.

Strategy: pure data-parallel over the batch axis (4 batches per core, no
collectives). Per core, per batch:
  - Q^T/K^T projections run in fp8(e4m3) DoubleRow mode: contraction 256 per
    pass (2x fewer PE matmuls). fp8 noise only perturbs attention logits
    (sigma~0.25) so end-to-end rel-err stays ~1.3e-2 (gate 2e-2). Weights are
    pre-scaled x32 on the host; the exp() activation scale divides it back out.
  - V and the output projection stay bf16 (their noise hits the output
    directly).
  - K^T is stored pair-packed: head 2m on partitions 0:64, head 2m+1 on
    64:128. scores^T then runs as K=64 row-tiled matmul PAIRS
    (tile_position (0,0)/(64,0)) -- two heads stream concurrently through
    disjoint row-groups of the PE array, ~2x scores throughput, and no
    zero-padding matmul waste.
  - scores^T packs the 4 causal diagonal blocks contiguously at [0,512) so
    the 0/1 triangular mask is ONE gpsimd multiply per head (not 4).
  - softmax without max-subtraction (logits bounded); attn@[V|1] accumulates
    all 4 t-chunks of a head into a single PSUM bank, so normalization is one
    batched reciprocal + one broadcast tensor_mul per head (stride-0 AP).
  - head-concat transpose via one batched DMA-transpose per t-chunk; final
    projection with bias folded in via a K=128 matmul; fp32 output.
"""

import sys

if "/opt/trn_rl_repo" not in sys.path:
    sys.path.insert(0, "/opt/trn_rl_repo")

import numpy as np
import ml_dtypes

B, T, C = 32, 512, 1024
H, D = 16, 64
HD = H * D
NCORES = 8
B_LOC = B // NCORES
SW = 32.0  # host-side prescale of wq/wk before fp8 cast

_CACHE = {}


def build_nc(b_loc=B_LOC):
    import concourse.mybir as mybir
    from concourse import bacc
    from concourse.bass import ds, ts
    from concourse.tile import TileContext

    f32 = mybir.dt.float32
    bf16 = mybir.dt.bfloat16
    f8 = mybir.dt.float8e4
    AF = mybir.ActivationFunctionType
    DR = mybir.MatmulPerfMode.DoubleRow

    KO = C // 128  # 8 contraction chunks
    KO2 = KO // 2  # 4 DoubleRow chunks (K=256 each)
    MO = HD // 128  # 8 output-row chunks
    TCH = T // 128  # 4 t-chunks
    EXP_SCALE = 1.0 / (float(np.sqrt(C)) * SW * SW)

    # scores^T causal packing, diagonal-blocks-first:
    #   cols [128j, 128j+128)    : diagonal block of s-chunk j  (j=0..3)
    #   cols [OD[j], OD[j]+ODW[j]): off-diagonal strip of s-chunk j covering
    #                              t in [128(j+1), T)           (j=0..2)
    # Bank layout (2KB fp32 = 512 cols): [0,512) bank0; [512,896)+[896,1024)
    # bank1; [1024,1280) bank2 -- no matmul output crosses a bank boundary.
    OD = [512, 1024, 896]
    ODW = [384, 256, 128]
    PACK = 1280

    def av_block(i, j):
        # column offset of the aT block for (t-chunk i, s-chunk j), j<=i
        return 128 * i if i == j else OD[j] + 128 * (i - j - 1)

    nc = bacc.Bacc("TRN2", target_bir_lowering=False)
    xT = nc.dram_tensor("xT", [b_loc, C, T], bf16, kind="ExternalInput")
    xT8 = nc.dram_tensor("xT8", [b_loc, C, T], f8, kind="ExternalInput")
    wq8 = nc.dram_tensor("wq8", [C, HD], f8, kind="ExternalInput")
    wk8 = nc.dram_tensor("wk8", [C, HD], f8, kind="ExternalInput")
    wv = nc.dram_tensor("wv", [C, HD], bf16, kind="ExternalInput")
    wp = nc.dram_tensor("wp", [C, C], bf16, kind="ExternalInput")
    bp = nc.dram_tensor("bp", [1, C], bf16, kind="ExternalInput")
    mask4 = nc.dram_tensor("mask4", [128, 512], bf16, kind="ExternalInput")
    out = nc.dram_tensor("out", [b_loc, T, C], f32, kind="ExternalOutput")

    with TileContext(nc) as tc:
        with (
            tc.tile_pool(name="weights", bufs=1) as wpool,
            tc.tile_pool(name="acts", bufs=2) as xpool,
            tc.tile_pool(name="attn", bufs=4) as apool,
            tc.tile_pool(name="small", bufs=8) as spool,
            tc.tile_pool(name="outs", bufs=2) as opool,
            tc.tile_pool(name="psS", bufs=2, space="PSUM") as psA,
            tc.tile_pool(name="ps1", bufs=2, space="PSUM") as psB,
        ):
            # ---- persistent weights ----
            # DMA order matters for the pipeline head: tiny tensors first,
            # then batch 0's fp8 x^T interleaved with wq8 so the first Q
            # matmul chain starts as soon as its operands land.
            wq8_sb = wpool.tile([128, KO, HD], f8, name="wq8_sb")
            wk8_sb = wpool.tile([128, KO, HD], f8, name="wk8_sb")
            wv_sb = wpool.tile([128, KO, HD], bf16, name="wv_sb")
            wp_sb = wpool.tile([128, KO, C], bf16, name="wp_sb")
            xT80_sb = xpool.tile([128, KO, T], f8, name="xT80_sb", tag="xT8")
            xT0_sb = xpool.tile([128, KO, T], bf16, name="xT0_sb", tag="xT")
            bp1_sb = wpool.tile([1, C], bf16, name="bp1_sb")
            nc.sync.dma_start(out=bp1_sb, in_=bp[:])
            mask4_sb = wpool.tile([128, 512], bf16, name="mask4_sb")
            nc.sync.dma_start(out=mask4_sb, in_=mask4[:])
            for k in range(KO):
                nc.sync.dma_start(out=xT80_sb[:, k, :], in_=xT8[0, ds(128 * k, 128), :])
                nc.sync.dma_start(out=wq8_sb[:, k, :], in_=wq8[ds(128 * k, 128), :])
            for k in range(KO):
                nc.sync.dma_start(out=wk8_sb[:, k, :], in_=wk8[ds(128 * k, 128), :])
            for k in range(KO):
                nc.sync.dma_start(out=xT0_sb[:, k, :], in_=xT[0, ds(128 * k, 128), :])
                nc.sync.dma_start(out=wv_sb[:, k, :], in_=wv[ds(128 * k, 128), :])
            nc.sync.dma_start(
                out=wp_sb, in_=wp[:].rearrange("(ko p) n -> p ko n", p=128)
            )
            # bias broadcast to all 128 partitions, f32, built once
            ones1_sb = wpool.tile([1, 128], bf16, name="ones1_sb")
            nc.gpsimd.memset(ones1_sb, 1.0)
            bias_bc = wpool.tile([128, C], f32, name="bias_bc")
            for half in range(2):
                psb = psB.tile([128, 512], f32, name="psb", tag="ps1")
                nc.tensor.matmul(
                    psb, ones1_sb, bp1_sb[:, ts(half, 512)], start=True, stop=True
                )
                nc.vector.tensor_copy(out=bias_bc[:, ts(half, 512)], in_=psb)
            # K^T pair-packed: [p, m, s] holds head 2m rows d=p on partitions
            # 0:64 and head 2m+1 rows d=p-64 on 64:128. Two persistent slots
            # for cross-batch overlap.
            kT2_tiles = [
                wpool.tile([128, MO, T], bf16, name=f"kT2_{slot}") for slot in range(2)
            ]

            for b in range(b_loc):
                kT2 = kT2_tiles[b % 2]
                # ---- load x^T (fp8 for QK, bf16 for V) for this batch ----
                if b == 0:
                    xT8_sb, xT_sb = xT80_sb, xT0_sb
                else:
                    xT8_sb = xpool.tile([128, KO, T], f8, name="xT8_sb", tag="xT8")
                    xT_sb = xpool.tile([128, KO, T], bf16, name="xT_sb", tag="xT")
                    for k in range(KO):
                        nc.sync.dma_start(
                            out=xT8_sb[:, k, :], in_=xT8[b, ds(128 * k, 128), :]
                        )
                    for k in range(KO):
                        nc.sync.dma_start(
                            out=xT_sb[:, k, :], in_=xT[b, ds(128 * k, 128), :]
                        )

                # ---- Q^T projection: [HD, T] fp8 DoubleRow ----
                qT_sb = xpool.tile([128, MO, T], bf16, name="qT_sb", tag="qT")
                for m in range(MO):
                    ps = psB.tile([128, T], f32, name="ps_q", tag="ps1")
                    for k2 in range(KO2):
                        nc.tensor.matmul(
                            ps,
                            wq8_sb[:, 2 * k2 : 2 * k2 + 2, ts(m, 128)],
                            xT8_sb[:, 2 * k2 : 2 * k2 + 2, :],
                            start=(k2 == 0),
                            stop=(k2 == KO2 - 1),
                            perf_mode=DR,
                        )
                    nc.scalar.copy(out=qT_sb[:, m, :], in_=ps)

                # ---- K^T projection (fp8 DoubleRow) into pair-packed tile ----
                for m in range(MO):
                    ps = psB.tile([128, T], f32, name="ps_k", tag="ps1")
                    for k2 in range(KO2):
                        nc.tensor.matmul(
                            ps,
                            wk8_sb[:, 2 * k2 : 2 * k2 + 2, ts(m, 128)],
                            xT8_sb[:, 2 * k2 : 2 * k2 + 2, :],
                            start=(k2 == 0),
                            stop=(k2 == KO2 - 1),
                            perf_mode=DR,
                        )
                    nc.vector.tensor_copy(out=kT2[:, m, :], in_=ps)

                # ---- V: [s, h, 65] with ones column at d=64 ----
                v_sb = xpool.tile([128, TCH, H, 65], bf16, name="v_sb", tag="v")
                nc.vector.memset(v_sb[:, :, :, 64:65], 1.0)
                for i in range(TCH):
                    for half in range(2):
                        ps = psB.tile([128, 512], f32, name="ps_v", tag="ps1")
                        for k in range(KO):
                            nc.tensor.matmul(
                                ps,
                                xT_sb[:, k, ts(i, 128)],
                                wv_sb[:, k, ts(half, 512)],
                                start=(k == 0),
                                stop=(k == KO - 1),
                            )
                        nc.scalar.copy(
                            out=v_sb[:, i, 8 * half : 8 * half + 8, 0:64],
                            in_=ps.rearrange("p (h d) -> p h d", d=64),
                        )

                # ---- attention, one head-pair at a time ----
                outT_sb = opool.tile([128, MO, T], bf16, name="outT_sb", tag="outT")
                on4 = opool.tile([128, TCH, HD], bf16, name="on4", tag="on4")
                for m in range(MO):
                    # scores^T for heads (2m, 2m+1) as K=64 row-tiled pairs:
                    # head 2m streams through PE rows 0:63, head 2m+1 through
                    # rows 64:127, concurrently.
                    psS = [
                        psA.tile([128, PACK], f32, name=f"psS{half}", tag="psS")
                        for half in range(2)
                    ]
                    for j in range(TCH):
                        for half in range(2):
                            nc.tensor.matmul(
                                psS[half][:, ds(128 * j, 128)],
                                kT2[ds(64 * half, 64), m, ds(128 * j, 128)],
                                qT_sb[ds(64 * half, 64), m, ds(128 * j, 128)],
                                start=True,
                                stop=True,
                                tile_position=(64 * half, 0),
                            )
                        if j < TCH - 1:
                            for half in range(2):
                                nc.tensor.matmul(
                                    psS[half][:, ds(OD[j], ODW[j])],
                                    kT2[ds(64 * half, 64), m, ds(128 * j, 128)],
                                    qT_sb[ds(64 * half, 64), m, ds(128 * (j + 1), ODW[j])],
                                    start=True,
                                    stop=True,
                                    tile_position=(64 * half, 0),
                                )
                    for half in range(2):
                        h = 2 * m + half
                        aT = apool.tile([128, PACK], bf16, name="aT", tag="aT")
                        nc.scalar.activation(aT, psS[half], AF.Exp, scale=EXP_SCALE)
                        # zero the masked (s>t) part of all 4 diagonal blocks
                        # in one multiply (they're packed contiguously)
                        nc.gpsimd.tensor_mul(
                            aT[:, 0:512], aT[:, 0:512], mask4_sb
                        )
                        # attn @ [V | 1]: all 4 t-chunks accumulate into ONE
                        # PSUM bank; col 64 of each chunk = softmax row-sum
                        psAV = psB.tile([128, TCH, 65], f32, name="psAV", tag="ps1")
                        for i in range(TCH):
                            for j in range(i + 1):
                                nc.tensor.matmul(
                                    psAV[:, i, :],
                                    aT[:, ds(av_block(i, j), 128)],
                                    v_sb[:, j, h, :],
                                    start=(i == 0 and j == 0),
                                    stop=(i == TCH - 1 and j == i),
                                )
                        rr = spool.tile([128, TCH], f32, name="rr", tag="rr")
                        nc.vector.reciprocal(rr, psAV[:, :, 64])
                        nc.vector.tensor_mul(
                            on4[:, :, ds(64 * h, 64)],
                            psAV[:, :, 0:64],
                            rr.broadcast_to([128, TCH, 64]),
                        )
                # batched head-concat transpose: [t, hd] -> [hd, t] per t-chunk
                for i in range(TCH):
                    nc.scalar.dma_start_transpose(
                        out=outT_sb[:, :, ts(i, 128)], in_=on4[:, i, :]
                    )

                # ---- final projection; bias added during PSUM evacuation ----
                for i in range(TCH):
                    out_sb = opool.tile([128, C], f32, name="out_sb", tag="out_sb")
                    for half in range(2):
                        # use the big scores-pool slots: keeps the small pool
                        # free for AV/QKV chains during phase overlap
                        psF = psA.tile([128, 512], f32, name="psF", tag="psS")
                        for k in range(MO):
                            nc.tensor.matmul(
                                psF,
                                outT_sb[:, k, ts(i, 128)],
                                wp_sb[:, k, ts(half, 512)],
                                start=(k == 0),
                                stop=(k == MO - 1),
                            )
                        nc.vector.tensor_add(
                            out=out_sb[:, ts(half, 512)],
                            in0=psF,
                            in1=bias_bc[:, ts(half, 512)],
                        )
                    nc.sync.dma_start(out=out[b, ts(i, 128), :], in_=out_sb)

    nc.compile()
    return nc


def make_in_maps(x, wq, wk, wv, w_proj, b_proj, b_loc=B_LOC, ncores=NCORES):
    bf16 = ml_dtypes.bfloat16
    f8 = ml_dtypes.float8_e4m3
    x = np.asarray(x, dtype=np.float32)
    # host-side layout prep (transpose / reshape / cast only)
    xTf = np.ascontiguousarray(x.transpose(0, 2, 1))  # [B, C, T] f32
    xT = xTf.astype(bf16)
    xT8 = xTf.astype(f8)
    wq2 = np.ascontiguousarray(
        np.asarray(wq, np.float32).transpose(1, 0, 2).reshape(C, HD)
    )
    wk2 = np.ascontiguousarray(
        np.asarray(wk, np.float32).transpose(1, 0, 2).reshape(C, HD)
    )
    wq8 = (wq2 * SW).astype(f8)
    wk8 = (wk2 * SW).astype(f8)
    wv2 = np.ascontiguousarray(
        np.asarray(wv, np.float32).transpose(1, 0, 2).reshape(C, HD)
    ).astype(bf16)
    wp2 = np.ascontiguousarray(np.asarray(w_proj, np.float32)).astype(bf16)
    bp2 = np.asarray(b_proj, np.float32).reshape(1, C).astype(bf16)
    # mask[p, f] = 1 where p <= f%128 (valid: s_in <= t_in on diagonal
    # blocks), tiled 4x horizontally for the packed diagonal region
    m1 = np.triu(np.ones((128, 128), np.float32))
    mask4 = np.tile(m1, (1, 4)).astype(bf16)
    in_maps = []
    for c in range(ncores):
        in_maps.append(
            {
                "xT": xT[c * b_loc : (c + 1) * b_loc],
                "xT8": xT8[c * b_loc : (c + 1) * b_loc],
                "wq8": wq8,
                "wk8": wk8,
                "wv": wv2,
                "wp": wp2,
                "bp": bp2,
                "mask4": mask4,
            }
        )
    return in_maps


def kernel(x, wq, wk, wv, w_proj, b_proj, **run_kwargs):
    from concourse import bass_utils

    if "nc" not in _CACHE:
        _CACHE["nc"] = build_nc(B_LOC)
    nc = _CACHE["nc"]
    in_maps = make_in_maps(x, wq, wk, wv, w_proj, b_proj)
    res = bass_utils.run_bass_kernel_spmd(
        nc, in_maps, core_ids=list(range(NCORES)), **run_kwargs
    )
    outs = [r["out"] for r in res.results]
    full = np.concatenate(outs, axis=0).astype(np.float32)
    if run_kwargs:
        _CACHE["last_result"] = res
    return full
